# revision 16
# baseline (speedup 1.0000x reference)
"""BiSSM global block on 8 Trainium2 NeuronCores.

Strategy (data-parallel over batch, one batch element per core):
  - Feature dims on SBUF partitions ("transposed" layout), time on the free
    axis: every einsum is a stationary-weight matmul with a wide moving dim.
  - The bidirectional linear recurrence h_t = h_{t-1} @ A.T + u_t has fast
    decaying memory (||(A.T)^k|| ~ 3e-3 at k=64): each direction is cut into
    S = N/K chunks of K steps, warmed up from zero over W extra steps.  All
    2S chunks advance together, so one scan step is a (H,H) matmul with a
    2S-wide moving operand - the whole scan is W+K matmul rounds instead of
    2N vector-matrix products.
  - u and y live in a (phase, chunk) factored layout: the per-step strided
    gathers become contiguous runs for the vector engine.
  - Backward scan relabels chunks c' = S-1-c so all reversed accesses are
    forward-strided in the same u buffer.
  - Matmul inputs bf16 (fp32 PSUM accumulate); y accumulated fp32.
  - Host prep (cheap, O(H^2)): assemble A, transpose weights, fold RMS
    `scale` and sqrt(D) into Bm/gate_w, and 0.5*alpha into Cm.
"""
import numpy as np
import ml_dtypes

B, N, D = 8, 2048, 768
H = 768
PT = 128          # SBUF partitions
KT = H // PT      # 6 feature tiles
K = 32            # chunk (emit) length
W = 64            # warmup window (must be 2*K)
S = N // K        # 64 chunks per direction
CC = (N + 2 * W) // K   # 68 chunk-columns in padded u (2 pad + 64 + 2 pad)
NBLK = 512        # time-block for projections
NB = N // NBLK    # 4 blocks
EPS = 1e-8

_CACHE = {}


def _build_nc():
    import concourse.tile as tile
    from concourse import bacc, mybir
    from contextlib import ExitStack

    f32 = mybir.dt.float32
    bf16 = mybir.dt.bfloat16
    ACT = mybir.ActivationFunctionType

    nc = bacc.Bacc("TRN2", target_bir_lowering=False, debug=False, num_devices=8)

    xT_d = nc.dram_tensor("xT", [KT, PT, N], f32, kind="ExternalInput").ap()
    Mw_d = nc.dram_tensor("Mw", [KT, PT, H], bf16, kind="ExternalInput").ap()
    M2_d = nc.dram_tensor("M2", [KT, PT, H], bf16, kind="ExternalInput").ap()
    MC_d = nc.dram_tensor("MC", [KT, PT, D], bf16, kind="ExternalInput").ap()
    BC_d = nc.dram_tensor("BC", [KT, PT, D], bf16, kind="ExternalInput").ap()
    CmT_d = nc.dram_tensor("CmT", [KT, PT, D], bf16, kind="ExternalInput").ap()
    BmT_d = nc.dram_tensor("BmT", [KT, PT, H], bf16, kind="ExternalInput").ap()
    gwT_d = nc.dram_tensor("gwT", [KT, PT, D], bf16, kind="ExternalInput").ap()
    gb_d = nc.dram_tensor("gb", [PT, KT], f32, kind="ExternalInput").ap()
    outT_d = nc.dram_tensor("outT", [KT, PT, N], f32, kind="ExternalOutput").ap()

    with tile.TileContext(nc) as tc:
        with ExitStack() as ctx:
            persist = ctx.enter_context(tc.tile_pool(name="persist", bufs=1))
            stream = ctx.enter_context(tc.tile_pool(name="stream", bufs=2))
            scr1 = ctx.enter_context(tc.tile_pool(name="scr1", bufs=1))
            rpool = ctx.enter_context(tc.tile_pool(name="rpool", bufs=2))

            # ---- persistent SBUF ----
            # u2[p, j, phase, cc] <-> padded-u column cc*K + phase
            u2 = persist.tile([PT, KT, K, CC], bf16)
            g_sb = persist.tile([PT, KT, N], bf16)
            # y2[p, j, phase, c] <-> y column c*K + phase
            y2 = persist.tile([PT, KT, K, S], f32)
            gb_s = persist.tile([PT, KT], f32)
            ones_s = persist.tile([PT, PT], bf16)
            eps_s = persist.tile([PT, 1], f32)

            nc.gpsimd.dma_start(out=gb_s, in_=gb_d)
            nc.vector.memset(ones_s, 1.0)
            nc.vector.memset(eps_s, D * EPS)
            # zero u padding: chunk-cols [0,2) and [CC-2, CC)
            nc.vector.memset(u2[:, :, :, 0:2], 0.0)
            nc.vector.memset(u2[:, :, :, CC - 2:CC], 0.0)

            # ---- phase 1: RMS norm, u-projection, gate (per 512-block) ----
            with tc.tile_pool(name="ph1psum", bufs=2, space="PSUM") as ph1p, \
                 tc.tile_pool(name="wph1", bufs=1) as wph1:
                BmT_s = wph1.tile([PT, KT, H], bf16)
                gwT_s = wph1.tile([PT, KT, D], bf16)
                BC_s = wph1.tile([PT, KT, D], bf16)
                for k in range(KT):
                    nc.gpsimd.dma_start(out=BmT_s[:, k, :], in_=BmT_d[k])
                    nc.gpsimd.dma_start(out=gwT_s[:, k, :], in_=gwT_d[k])
                    nc.gpsimd.dma_start(out=BC_s[:, k, :], in_=BC_d[k])
                for nb in range(NB):
                    cols = slice(nb * NBLK, (nb + 1) * NBLK)
                    xb = stream.tile([PT, KT, NBLK], f32, tag="xb")
                    nc.gpsimd.dma_start(
                        out=xb, in_=xT_d[:, :, cols].transpose([1, 0, 2]))
                    # sumsq over d via all-ones matmul (broadcasts to all parts)
                    psR = ph1p.tile([PT, NBLK], f32, tag="psR")
                    sq = scr1.tile([PT, KT, NBLK], bf16, tag="sq")
                    for k in range(KT):
                        nc.vector.tensor_mul(sq[:, k, :], xb[:, k, :], xb[:, k, :])
                        nc.tensor.matmul(psR, ones_s, sq[:, k, :],
                                         start=(k == 0), stop=(k == KT - 1))
                    # rb = sqrt(sumsq + D*eps); 1/sqrt(D) folded into Bm/gw
                    rb = stream.tile([PT, NBLK], f32, tag="rb")
                    nc.scalar.activation(out=rb, in_=psR, func=ACT.Sqrt,
                                         bias=eps_s, scale=1.0)
                    nc.vector.reciprocal(out=rb, in_=rb)
                    xn = stream.tile([PT, KT, NBLK], bf16, tag="xn")
                    for k in range(KT):
                        nc.vector.tensor_mul(xn[:, k, :], xb[:, k, :], rb)
                    # u block -> u2 chunk-cols [nb*16+2, nb*16+18)
                    cc0 = nb * (NBLK // K) + W // K
                    for j in range(KT):
                        psU = ph1p.tile([PT, NBLK], f32, tag="psU")
                        for k in range(KT):
                            nc.tensor.matmul(psU, BmT_s[:, k, j * PT:(j + 1) * PT],
                                             xn[:, k, :],
                                             start=(k == 0), stop=(k == KT - 1))
                        dest = u2[:, j, :, cc0:cc0 + NBLK // K].transpose([0, 2, 1])
                        if j % 2 == 0:
                            nc.vector.tensor_copy(dest, psU)
                        else:
                            nc.scalar.activation(out=dest, in_=psU, func=ACT.Copy)
                    # UC = xn @ (Bm' Cm^T a/2): initializes y2 (the one
                    # MC-emitted contribution per column carries this term)
                    for j in range(KT):
                        psUC = ph1p.tile([PT, NBLK], f32, tag="psUC")
                        for k in range(KT):
                            nc.tensor.matmul(psUC, BC_s[:, k, j * PT:(j + 1) * PT],
                                             xn[:, k, :],
                                             start=(k == 0), stop=(k == KT - 1))
                        ydest = y2[:, j, :, nb * (NBLK // K):(nb + 1) * (NBLK // K)]
                        ydest = ydest.transpose([0, 2, 1])
                        if j % 2 == 0:
                            nc.scalar.activation(out=ydest, in_=psUC, func=ACT.Copy)
                        else:
                            nc.vector.tensor_copy(ydest, psUC)
                    # gate
                    for j in range(KT):
                        psG = ph1p.tile([PT, NBLK], f32, tag="psG")
                        for k in range(KT):
                            nc.tensor.matmul(psG, gwT_s[:, k, j * PT:(j + 1) * PT],
                                             xn[:, k, :],
                                             start=(k == 0), stop=(k == KT - 1))
                        nc.scalar.activation(out=g_sb[:, j, cols], in_=psG,
                                             func=ACT.Sigmoid, bias=gb_s[:, j:j + 1])

            # ---- phase 1.5: v = u@M; merge w into u2 (pair-step inputs) ----
            # fwd pair input at even padded col t: w[t] = v[t-1] + u[t]
            # bwd pair input at odd  padded col t: w~[t] = v[t+1] + u[t]
            # phases 0 and 31 stay raw in u2 (R0 init); their merged values
            # live in side buffers w0 / w31.
            w0 = persist.tile([PT, KT, CC], bf16)
            w31 = persist.tile([PT, KT, CC], bf16)
            NPH = 4   # phases per v-matmul group
            with tc.tile_pool(name="vstage", bufs=1) as vst, \
                 tc.tile_pool(name="vpsum", bufs=2, space="PSUM") as vps:
                Mw_s = vst.tile([PT, KT, H], bf16)
                for k in range(KT):
                    nc.gpsimd.dma_start(out=Mw_s[:, k, :], in_=Mw_d[k])
                vbuf = vst.tile([PT, KT, K, CC], bf16)
                for gph in range(K // NPH):
                    phs = slice(gph * NPH, (gph + 1) * NPH)
                    for j in range(KT):
                        psV = vps.tile([PT, NPH, CC], f32, tag="psV")
                        for k in range(KT):
                            nc.tensor.matmul(psV, Mw_s[:, k, j * PT:(j + 1) * PT],
                                             u2[:, k, phs, :],
                                             start=(k == 0), stop=(k == KT - 1))
                        nc.scalar.activation(out=vbuf[:, j, phs, :], in_=psV,
                                             func=ACT.Copy)
                # merges (in-place into u2 for phases 1..30)
                for j in range(KT):
                    # even phases 2..30: w = v[p-1] + u[p]
                    tgt = u2[:, j, 2:K - 1:2, :]
                    nc.vector.tensor_add(tgt, vbuf[:, j, 1:K - 2:2, :], tgt)
                    # odd phases 1..29: w~ = v[p+1] + u[p]
                    tgt = u2[:, j, 1:K - 2:2, :]
                    nc.vector.tensor_add(tgt, vbuf[:, j, 2:K - 1:2, :], tgt)
                    # w0[cc] = v[31, cc-1] + u[0, cc]
                    nc.vector.memset(w0[:, j, 0:1], 0.0)
                    nc.vector.tensor_add(w0[:, j, 1:CC], vbuf[:, j, K - 1, 0:CC - 1],
                                         u2[:, j, 0, 1:CC])
                    # w31[cc] = v[0, cc+1] + u[31, cc]
                    nc.vector.memset(w31[:, j, CC - 1:CC], 0.0)
                    nc.vector.tensor_add(w31[:, j, 0:CC - 1], vbuf[:, j, 0, 1:CC],
                                         u2[:, j, K - 1, 0:CC - 1])

            # ---- phase 2: paired (M^2) bidirectional recurrence ----
            # States at even local steps only.  Round to state s (even):
            #   fwd chunk c consumes slot col c*K + s, bwd col c*K + (2W+K-1-s)
            # Emits (s >= W, i = s-W even):
            #   direct (Cm^T): fwd -> y2 phase i,   bwd -> phase K-1-i
            #   MC (M Cm^T):   fwd -> y2 phase i+1, bwd -> phase K-2-i
            with tc.tile_pool(name="ph2psum", bufs=2, space="PSUM") as ph2p, \
                 tc.tile_pool(name="ph2psumC", bufs=4, space="PSUM") as ph2pc, \
                 tc.tile_pool(name="wph2", bufs=1) as wph2:
                M2_s = wph2.tile([PT, KT, H], bf16)
                CmT_s = wph2.tile([PT, KT, D], bf16)
                MC_s = wph2.tile([PT, KT, D], bf16)
                for k in range(KT):
                    nc.gpsimd.dma_start(out=M2_s[:, k, :], in_=M2_d[k])
                    nc.gpsimd.dma_start(out=CmT_s[:, k, :], in_=CmT_d[k])
                    nc.gpsimd.dma_start(out=MC_s[:, k, :], in_=MC_d[k])
                R_prev = rpool.tile([PT, KT, 2 * S], bf16, tag="R")
                nc.vector.tensor_copy(R_prev[:, :, 0:S], u2[:, :, 0, 0:S])
                nc.vector.tensor_copy(R_prev[:, :, S:2 * S],
                                      u2[:, :, K - 1, (2 * W) // K:(2 * W) // K + S])
                for s in range(2, W + K, 2):
                    q, r = divmod(s, K)
                    fwd_u = w0[:, :, q:q + S] if r == 0 else u2[:, :, r, q:q + S]
                    cb = 2 * W + K - 1 - s
                    qb, rb_ = divmod(cb, K)
                    bwd_u = (w31[:, :, qb:qb + S] if rb_ == K - 1
                             else u2[:, :, rb_, qb:qb + S])
                    R_new = rpool.tile([PT, KT, 2 * S], bf16, tag="R")
                    psA4 = ph2p.tile([PT, 4, 2 * S], f32, tag="psA4")
                    psA2 = ph2p.tile([PT, 2, 2 * S], f32, tag="psA2")
                    for j in range(KT):
                        ps = psA4[:, j, :] if j < 4 else psA2[:, j - 4, :]
                        for k in range(KT):
                            nc.tensor.matmul(ps, M2_s[:, k, j * PT:(j + 1) * PT],
                                             R_prev[:, k, :],
                                             start=(k == 0), stop=(k == KT - 1))
                    nc.vector.tensor_add(R_new[:, 0:4, 0:S], psA4[:, :, 0:S],
                                         fwd_u[:, 0:4, :])
                    nc.vector.tensor_add(R_new[:, 0:4, S:2 * S],
                                         psA4[:, :, S:2 * S], bwd_u[:, 0:4, :])
                    nc.vector.tensor_add(R_new[:, 4:6, 0:S], psA2[:, :, 0:S],
                                         fwd_u[:, 4:6, :])
                    nc.vector.tensor_add(R_new[:, 4:6, S:2 * S],
                                         psA2[:, :, S:2 * S], bwd_u[:, 4:6, :])
                    if s >= W:
                        i = s - W
                        for wt, pf, pb in ((CmT_s, i, K - 1 - i),
                                           (MC_s, i + 1, K - 2 - i)):
                            for dg in range(2):
                                psC = ph2pc.tile([PT, 3, 2 * S], f32, tag="psC")
                                for dd in range(3):
                                    j = dg * 3 + dd
                                    for k in range(KT):
                                        nc.tensor.matmul(
                                            psC[:, dd, :],
                                            wt[:, k, j * PT:(j + 1) * PT],
                                            R_new[:, k, :],
                                            start=(k == 0), stop=(k == KT - 1))
                                yf = y2[:, dg * 3:(dg + 1) * 3, pf, :]
                                nc.vector.tensor_add(yf, yf, psC[:, :, 0:S])
                                yb = y2[:, dg * 3:(dg + 1) * 3, pb, :]
                                nc.vector.tensor_add(yb, yb, psC[:, :, S:2 * S])
                    R_prev = R_new

            # ---- phase 3: out = x + y*g  (y pre-scaled by 0.5*alpha) ----
            for nb in range(NB):
                cols = slice(nb * NBLK, (nb + 1) * NBLK)
                xb = stream.tile([PT, KT, NBLK], f32, tag="xb")
                for k in range(KT):
                    nc.gpsimd.dma_start(out=xb[:, k, :], in_=xT_d[k, :, cols])
                # y2 block in t-order: (cc-major, phase-minor)
                c0 = nb * (NBLK // K)
                ysl = y2[:, :, :, c0:c0 + NBLK // K].transpose([0, 1, 3, 2])
                yg = scr1.tile([PT, KT, NBLK // K, K], f32, tag="yg")
                gsl = g_sb[:, :, cols].rearrange("p j (c f) -> p j c f", f=K)
                nc.vector.tensor_mul(yg, ysl, gsl)
                nc.vector.tensor_add(yg, yg, xb.rearrange("p j (c f) -> p j c f", f=K))
                for k in range(KT):
                    nc.gpsimd.dma_start(
                        out=outT_d[k, :, cols],
                        in_=yg[:, k, :, :].rearrange("p c f -> p (c f)"))

    nc.compile()
    return nc


def _get_nc():
    if "nc" not in _CACHE:
        _CACHE["nc"] = _build_nc()
    return _CACHE["nc"]


def _prep_maps(x, scale, U, V, S_param, Bm, Cm, gate_w, gate_b, alpha):
    bf = ml_dtypes.bfloat16
    A_diag = -np.linspace(1.0, float(H), H, dtype=np.float32) / H
    A = np.diag(A_diag) + U @ V.T + (S_param - S_param.T)
    M = np.ascontiguousarray(A.T)                       # (h_in, h_out)
    CmT = np.ascontiguousarray(Cm.T) * (0.5 * float(alpha[0]))
    sD = np.sqrt(float(D)).astype(np.float32)
    BmT = np.ascontiguousarray(Bm.T) * (scale * sD)[:, None]
    gwT = np.ascontiguousarray(gate_w.T) * (scale * sD)[:, None]

    def tiles(w):  # (H, F) -> (KT, PT, F)
        return np.ascontiguousarray(w.reshape(KT, PT, -1))

    M64 = M.astype(np.float64)
    M2 = (M64 @ M64).astype(np.float32)
    MC = (M64 @ CmT.astype(np.float64)).astype(np.float32)
    BC = (BmT.astype(np.float64) @ CmT.astype(np.float64)).astype(np.float32)
    Mw = tiles(M).astype(bf)
    M2s = tiles(M2).astype(bf)
    MCs = tiles(MC).astype(bf)
    BCs = tiles(BC).astype(bf)
    CmTs = tiles(CmT).astype(bf)
    BmTs = tiles(BmT).astype(bf)
    gwTs = tiles(gwT).astype(bf)
    gb = np.ascontiguousarray(gate_b.reshape(KT, PT).T).astype(np.float32)

    in_maps = []
    for b in range(B):
        xT = np.ascontiguousarray(x[b].T.reshape(KT, PT, N)).astype(np.float32)
        in_maps.append({"xT": xT, "Mw": Mw, "M2": M2s, "MC": MCs, "BC": BCs,
                        "CmT": CmTs, "BmT": BmTs, "gwT": gwTs, "gb": gb})
    return in_maps


def run(inputs, trace=False):
    from concourse.bass_utils import run_bass_kernel_spmd
    nc = _get_nc()
    in_maps = _prep_maps(**inputs)
    if trace:
        _install_ntff_hook()
    res = run_bass_kernel_spmd(nc, in_maps, core_ids=list(range(8)), trace=trace)
    out = np.empty((B, N, D), np.float32)
    for b in range(B):
        outT = np.asarray(res.results[b]["outT"]).reshape(H, N)
        out[b] = outT.T
    return out, res


def _install_ntff_hook():
    import sys, types
    try:
        from antenv.axon_hooks import get_axon_ntff_profile_hook  # noqa
        return
    except ImportError:
        pass
    try:
        from trn_agent_boot.trn_boot import _ntff_profile_via_ctypes
        hook = _ntff_profile_via_ctypes("/opt/axon/libaxon_pjrt.so")
    except Exception:
        hook = None
    mod = types.ModuleType("antenv.axon_hooks")
    mod._hook = hook
    mod.get_axon_ntff_profile_hook = lambda: mod._hook
    mod.set_axon_ntff_profile_hook = lambda h: setattr(mod, "_hook", h)
    sys.modules["antenv.axon_hooks"] = mod


def kernel(**inputs):
    out, _ = run(inputs, trace=False)
    return out


# revision 17
# speedup vs baseline: 1.0179x; 1.0179x over previous
"""BiSSM global block on 8 Trainium2 NeuronCores.

Strategy (data-parallel over batch, one batch element per core):
  - Feature dims on SBUF partitions ("transposed" layout), time on the free
    axis: every einsum is a stationary-weight matmul with a wide moving dim.
  - The bidirectional linear recurrence h_t = h_{t-1} @ A.T + u_t has fast
    decaying memory (||(A.T)^k|| ~ 3e-3 at k=64): each direction is cut into
    S = N/K chunks of K steps, warmed up from zero over W extra steps.  All
    2S chunks advance together, so one scan step is a (H,H) matmul with a
    2S-wide moving operand - the whole scan is W+K matmul rounds instead of
    2N vector-matrix products.
  - u and y live in a (phase, chunk) factored layout: the per-step strided
    gathers become contiguous runs for the vector engine.
  - Backward scan relabels chunks c' = S-1-c so all reversed accesses are
    forward-strided in the same u buffer.
  - Matmul inputs bf16 (fp32 PSUM accumulate); y accumulated fp32.
  - Host prep (cheap, O(H^2)): assemble A, transpose weights, fold RMS
    `scale` and sqrt(D) into Bm/gate_w, and 0.5*alpha into Cm.
"""
import numpy as np
import ml_dtypes

B, N, D = 8, 2048, 768
H = 768
PT = 128          # SBUF partitions
KT = H // PT      # 6 feature tiles
K = 32            # chunk (emit) length
W = 64            # warmup window (must be 2*K)
S = N // K        # 64 chunks per direction
CC = (N + 2 * W) // K   # 68 chunk-columns in padded u (2 pad + 64 + 2 pad)
NBLK = 512        # time-block for projections
NB = N // NBLK    # 4 blocks
EPS = 1e-8

_CACHE = {}


def _build_nc():
    import concourse.tile as tile
    from concourse import bacc, mybir
    from contextlib import ExitStack

    f32 = mybir.dt.float32
    bf16 = mybir.dt.bfloat16
    ACT = mybir.ActivationFunctionType

    nc = bacc.Bacc("TRN2", target_bir_lowering=False, debug=False, num_devices=8)

    xT_d = nc.dram_tensor("xT", [KT, PT, N], f32, kind="ExternalInput").ap()
    Mw_d = nc.dram_tensor("Mw", [KT, PT, H], bf16, kind="ExternalInput").ap()
    M2_d = nc.dram_tensor("M2", [KT, PT, H], bf16, kind="ExternalInput").ap()
    MC_d = nc.dram_tensor("MC", [KT, PT, D], bf16, kind="ExternalInput").ap()
    BC_d = nc.dram_tensor("BC", [KT, PT, D], bf16, kind="ExternalInput").ap()
    CmT_d = nc.dram_tensor("CmT", [KT, PT, D], bf16, kind="ExternalInput").ap()
    BmT_d = nc.dram_tensor("BmT", [KT, PT, H], bf16, kind="ExternalInput").ap()
    gwT_d = nc.dram_tensor("gwT", [KT, PT, D], bf16, kind="ExternalInput").ap()
    gb_d = nc.dram_tensor("gb", [PT, KT], f32, kind="ExternalInput").ap()
    outT_d = nc.dram_tensor("outT", [KT, PT, N], f32, kind="ExternalOutput").ap()

    with tile.TileContext(nc) as tc:
        with ExitStack() as ctx:
            persist = ctx.enter_context(tc.tile_pool(name="persist", bufs=1))
            stream = ctx.enter_context(tc.tile_pool(name="stream", bufs=2))
            scr1 = ctx.enter_context(tc.tile_pool(name="scr1", bufs=1))
            rpool = ctx.enter_context(tc.tile_pool(name="rpool", bufs=2))

            # ---- persistent SBUF ----
            # u2[p, j, phase, cc] <-> padded-u column cc*K + phase
            u2 = persist.tile([PT, KT, K, CC], bf16)
            g_sb = persist.tile([PT, KT, N], bf16)
            # y2[p, j, phase, c] <-> y column c*K + phase
            y2 = persist.tile([PT, KT, K, S], f32)
            gb_s = persist.tile([PT, KT], f32)
            ones_s = persist.tile([PT, PT], bf16)
            eps_s = persist.tile([PT, 1], f32)

            nc.gpsimd.dma_start(out=gb_s, in_=gb_d)
            nc.vector.memset(ones_s, 1.0)
            nc.vector.memset(eps_s, D * EPS)
            # zero u padding: chunk-cols [0,2) and [CC-2, CC)
            nc.vector.memset(u2[:, :, :, 0:2], 0.0)
            nc.vector.memset(u2[:, :, :, CC - 2:CC], 0.0)

            # ---- phase 1: RMS norm, u-projection, gate (per 512-block) ----
            with tc.tile_pool(name="ph1psum", bufs=2, space="PSUM") as ph1p, \
                 tc.tile_pool(name="wph1", bufs=1) as wph1:
                BmT_s = wph1.tile([PT, KT, H], bf16)
                gwT_s = wph1.tile([PT, KT, D], bf16)
                BC_s = wph1.tile([PT, KT, D], bf16)
                for k in range(KT):
                    nc.gpsimd.dma_start(out=BmT_s[:, k, :], in_=BmT_d[k])
                    nc.gpsimd.dma_start(out=gwT_s[:, k, :], in_=gwT_d[k])
                    nc.gpsimd.dma_start(out=BC_s[:, k, :], in_=BC_d[k])
                for nb in range(NB):
                    cols = slice(nb * NBLK, (nb + 1) * NBLK)
                    xb = stream.tile([PT, KT, NBLK], f32, tag="xb")
                    for k in range(KT):
                        nc.gpsimd.dma_start(out=xb[:, k, :], in_=xT_d[k, :, cols])
                    # sumsq over d via all-ones matmul (broadcasts to all parts)
                    psR = ph1p.tile([PT, NBLK], f32, tag="psR")
                    sq = scr1.tile([PT, KT, NBLK], bf16, tag="sq")
                    for k in range(KT):
                        nc.vector.tensor_mul(sq[:, k, :], xb[:, k, :], xb[:, k, :])
                        nc.tensor.matmul(psR, ones_s, sq[:, k, :],
                                         start=(k == 0), stop=(k == KT - 1))
                    # rb = sqrt(sumsq + D*eps); 1/sqrt(D) folded into Bm/gw
                    rb = stream.tile([PT, NBLK], f32, tag="rb")
                    nc.scalar.activation(out=rb, in_=psR, func=ACT.Sqrt,
                                         bias=eps_s, scale=1.0)
                    nc.vector.reciprocal(out=rb, in_=rb)
                    xn = stream.tile([PT, KT, NBLK], bf16, tag="xn")
                    for k in range(KT):
                        nc.vector.tensor_mul(xn[:, k, :], xb[:, k, :], rb)
                    # u block -> u2 chunk-cols [nb*16+2, nb*16+18)
                    cc0 = nb * (NBLK // K) + W // K
                    for j in range(KT):
                        psU = ph1p.tile([PT, NBLK], f32, tag="psU")
                        for k in range(KT):
                            nc.tensor.matmul(psU, BmT_s[:, k, j * PT:(j + 1) * PT],
                                             xn[:, k, :],
                                             start=(k == 0), stop=(k == KT - 1))
                        dest = u2[:, j, :, cc0:cc0 + NBLK // K].transpose([0, 2, 1])
                        if j % 2 == 0:
                            nc.vector.tensor_copy(dest, psU)
                        else:
                            nc.scalar.activation(out=dest, in_=psU, func=ACT.Copy)
                    # UC = xn @ (Bm' Cm^T a/2): initializes y2 (the one
                    # MC-emitted contribution per column carries this term)
                    for j in range(KT):
                        psUC = ph1p.tile([PT, NBLK], f32, tag="psUC")
                        for k in range(KT):
                            nc.tensor.matmul(psUC, BC_s[:, k, j * PT:(j + 1) * PT],
                                             xn[:, k, :],
                                             start=(k == 0), stop=(k == KT - 1))
                        ydest = y2[:, j, :, nb * (NBLK // K):(nb + 1) * (NBLK // K)]
                        ydest = ydest.transpose([0, 2, 1])
                        if j % 2 == 0:
                            nc.scalar.activation(out=ydest, in_=psUC, func=ACT.Copy)
                        else:
                            nc.vector.tensor_copy(ydest, psUC)
                    # gate
                    for j in range(KT):
                        psG = ph1p.tile([PT, NBLK], f32, tag="psG")
                        for k in range(KT):
                            nc.tensor.matmul(psG, gwT_s[:, k, j * PT:(j + 1) * PT],
                                             xn[:, k, :],
                                             start=(k == 0), stop=(k == KT - 1))
                        nc.scalar.activation(out=g_sb[:, j, cols], in_=psG,
                                             func=ACT.Sigmoid, bias=gb_s[:, j:j + 1])

            # ---- phase 1.5: v = u@M; merge w into u2 (pair-step inputs) ----
            # fwd pair input at even padded col t: w[t] = v[t-1] + u[t]
            # bwd pair input at odd  padded col t: w~[t] = v[t+1] + u[t]
            # phases 0 and 31 stay raw in u2 (R0 init); their merged values
            # live in side buffers w0 / w31.
            w0 = persist.tile([PT, KT, CC], bf16)
            w31 = persist.tile([PT, KT, CC], bf16)
            NPH = 4   # phases per v-matmul group
            with tc.tile_pool(name="vstage", bufs=1) as vst, \
                 tc.tile_pool(name="vpsum", bufs=2, space="PSUM") as vps:
                Mw_s = vst.tile([PT, KT, H], bf16)
                for k in range(KT):
                    nc.gpsimd.dma_start(out=Mw_s[:, k, :], in_=Mw_d[k])
                vbuf = vst.tile([PT, KT, K, CC], bf16)
                for gph in range(K // NPH):
                    phs = slice(gph * NPH, (gph + 1) * NPH)
                    for j in range(KT):
                        psV = vps.tile([PT, NPH, CC], f32, tag="psV")
                        for k in range(KT):
                            nc.tensor.matmul(psV, Mw_s[:, k, j * PT:(j + 1) * PT],
                                             u2[:, k, phs, :],
                                             start=(k == 0), stop=(k == KT - 1))
                        nc.scalar.activation(out=vbuf[:, j, phs, :], in_=psV,
                                             func=ACT.Copy)
                # merges (in-place into u2 for phases 1..30)
                for j in range(KT):
                    # even phases 2..30: w = v[p-1] + u[p]
                    tgt = u2[:, j, 2:K - 1:2, :]
                    nc.vector.tensor_add(tgt, vbuf[:, j, 1:K - 2:2, :], tgt)
                    # odd phases 1..29: w~ = v[p+1] + u[p]
                    tgt = u2[:, j, 1:K - 2:2, :]
                    nc.vector.tensor_add(tgt, vbuf[:, j, 2:K - 1:2, :], tgt)
                    # w0[cc] = v[31, cc-1] + u[0, cc]
                    nc.vector.memset(w0[:, j, 0:1], 0.0)
                    nc.vector.tensor_add(w0[:, j, 1:CC], vbuf[:, j, K - 1, 0:CC - 1],
                                         u2[:, j, 0, 1:CC])
                    # w31[cc] = v[0, cc+1] + u[31, cc]
                    nc.vector.memset(w31[:, j, CC - 1:CC], 0.0)
                    nc.vector.tensor_add(w31[:, j, 0:CC - 1], vbuf[:, j, 0, 1:CC],
                                         u2[:, j, K - 1, 0:CC - 1])

            # ---- phase 2: paired (M^2) bidirectional recurrence ----
            # States at even local steps only.  Round to state s (even):
            #   fwd chunk c consumes slot col c*K + s, bwd col c*K + (2W+K-1-s)
            # Emits (s >= W, i = s-W even):
            #   direct (Cm^T): fwd -> y2 phase i,   bwd -> phase K-1-i
            #   MC (M Cm^T):   fwd -> y2 phase i+1, bwd -> phase K-2-i
            with tc.tile_pool(name="ph2psum", bufs=2, space="PSUM") as ph2p, \
                 tc.tile_pool(name="ph2psumC", bufs=3, space="PSUM") as ph2pc, \
                 tc.tile_pool(name="wph2", bufs=1) as wph2:
                M2_s = wph2.tile([PT, KT, H], bf16)
                CmT_s = wph2.tile([PT, KT, D], bf16)
                MC_s = wph2.tile([PT, KT, D], bf16)
                for k in range(KT):
                    nc.gpsimd.dma_start(out=M2_s[:, k, :], in_=M2_d[k])
                    nc.gpsimd.dma_start(out=CmT_s[:, k, :], in_=CmT_d[k])
                    nc.gpsimd.dma_start(out=MC_s[:, k, :], in_=MC_d[k])
                R_prev = rpool.tile([PT, KT, 2 * S], bf16, tag="R")
                nc.vector.tensor_copy(R_prev[:, :, 0:S], u2[:, :, 0, 0:S])
                nc.vector.tensor_copy(R_prev[:, :, S:2 * S],
                                      u2[:, :, K - 1, (2 * W) // K:(2 * W) // K + S])
                for s in range(2, W + K, 2):
                    q, r = divmod(s, K)
                    fwd_u = w0[:, :, q:q + S] if r == 0 else u2[:, :, r, q:q + S]
                    cb = 2 * W + K - 1 - s
                    qb, rb_ = divmod(cb, K)
                    bwd_u = (w31[:, :, qb:qb + S] if rb_ == K - 1
                             else u2[:, :, rb_, qb:qb + S])
                    R_new = rpool.tile([PT, KT, 2 * S], bf16, tag="R")
                    psA4 = ph2p.tile([PT, 4, 2 * S], f32, tag="psA4")
                    psA2 = ph2p.tile([PT, 2, 2 * S], f32, tag="psA2")
                    for j in range(KT):
                        ps = psA4[:, j, :] if j < 4 else psA2[:, j - 4, :]
                        for k in range(KT):
                            nc.tensor.matmul(ps, M2_s[:, k, j * PT:(j + 1) * PT],
                                             R_prev[:, k, :],
                                             start=(k == 0), stop=(k == KT - 1))
                    nc.vector.tensor_add(R_new[:, 0:4, 0:S], psA4[:, :, 0:S],
                                         fwd_u[:, 0:4, :])
                    nc.vector.tensor_add(R_new[:, 0:4, S:2 * S],
                                         psA4[:, :, S:2 * S], bwd_u[:, 0:4, :])
                    nc.vector.tensor_add(R_new[:, 4:6, 0:S], psA2[:, :, 0:S],
                                         fwd_u[:, 4:6, :])
                    nc.vector.tensor_add(R_new[:, 4:6, S:2 * S],
                                         psA2[:, :, S:2 * S], bwd_u[:, 4:6, :])
                    if s >= W:
                        i = s - W
                        for wt, pf, pb in ((CmT_s, i, K - 1 - i),
                                           (MC_s, i + 1, K - 2 - i)):
                            for dg in range(2):
                                psC = ph2pc.tile([PT, 3, 2 * S], f32, tag="psC")
                                for dd in range(3):
                                    j = dg * 3 + dd
                                    for k in range(KT):
                                        nc.tensor.matmul(
                                            psC[:, dd, :],
                                            wt[:, k, j * PT:(j + 1) * PT],
                                            R_new[:, k, :],
                                            start=(k == 0), stop=(k == KT - 1))
                                yf = y2[:, dg * 3:(dg + 1) * 3, pf, :]
                                nc.vector.tensor_add(yf, yf, psC[:, :, 0:S])
                                yb = y2[:, dg * 3:(dg + 1) * 3, pb, :]
                                nc.vector.tensor_add(yb, yb, psC[:, :, S:2 * S])
                    R_prev = R_new

            # ---- phase 3: out = x + y*g  (y pre-scaled by 0.5*alpha) ----
            for nb in range(NB):
                cols = slice(nb * NBLK, (nb + 1) * NBLK)
                xb = stream.tile([PT, KT, NBLK], f32, tag="xb")
                for k in range(KT):
                    nc.gpsimd.dma_start(out=xb[:, k, :], in_=xT_d[k, :, cols])
                # y2 block in t-order: (cc-major, phase-minor)
                c0 = nb * (NBLK // K)
                ysl = y2[:, :, :, c0:c0 + NBLK // K].transpose([0, 1, 3, 2])
                yg = scr1.tile([PT, KT, NBLK // K, K], f32, tag="yg")
                gsl = g_sb[:, :, cols].rearrange("p j (c f) -> p j c f", f=K)
                nc.vector.tensor_mul(yg, ysl, gsl)
                nc.vector.tensor_add(yg, yg, xb.rearrange("p j (c f) -> p j c f", f=K))
                for k in range(KT):
                    nc.gpsimd.dma_start(
                        out=outT_d[k, :, cols],
                        in_=yg[:, k, :, :].rearrange("p c f -> p (c f)"))

    nc.compile()
    return nc


def _get_nc():
    if "nc" not in _CACHE:
        _CACHE["nc"] = _build_nc()
    return _CACHE["nc"]


def _prep_maps(x, scale, U, V, S_param, Bm, Cm, gate_w, gate_b, alpha):
    bf = ml_dtypes.bfloat16
    A_diag = -np.linspace(1.0, float(H), H, dtype=np.float32) / H
    A = np.diag(A_diag) + U @ V.T + (S_param - S_param.T)
    M = np.ascontiguousarray(A.T)                       # (h_in, h_out)
    CmT = np.ascontiguousarray(Cm.T) * (0.5 * float(alpha[0]))
    sD = np.sqrt(float(D)).astype(np.float32)
    BmT = np.ascontiguousarray(Bm.T) * (scale * sD)[:, None]
    gwT = np.ascontiguousarray(gate_w.T) * (scale * sD)[:, None]

    def tiles(w):  # (H, F) -> (KT, PT, F)
        return np.ascontiguousarray(w.reshape(KT, PT, -1))

    M64 = M.astype(np.float64)
    M2 = (M64 @ M64).astype(np.float32)
    MC = (M64 @ CmT.astype(np.float64)).astype(np.float32)
    BC = (BmT.astype(np.float64) @ CmT.astype(np.float64)).astype(np.float32)
    Mw = tiles(M).astype(bf)
    M2s = tiles(M2).astype(bf)
    MCs = tiles(MC).astype(bf)
    BCs = tiles(BC).astype(bf)
    CmTs = tiles(CmT).astype(bf)
    BmTs = tiles(BmT).astype(bf)
    gwTs = tiles(gwT).astype(bf)
    gb = np.ascontiguousarray(gate_b.reshape(KT, PT).T).astype(np.float32)

    in_maps = []
    for b in range(B):
        xT = np.ascontiguousarray(x[b].T.reshape(KT, PT, N)).astype(np.float32)
        in_maps.append({"xT": xT, "Mw": Mw, "M2": M2s, "MC": MCs, "BC": BCs,
                        "CmT": CmTs, "BmT": BmTs, "gwT": gwTs, "gb": gb})
    return in_maps


def run(inputs, trace=False):
    from concourse.bass_utils import run_bass_kernel_spmd
    nc = _get_nc()
    in_maps = _prep_maps(**inputs)
    if trace:
        _install_ntff_hook()
    res = run_bass_kernel_spmd(nc, in_maps, core_ids=list(range(8)), trace=trace)
    out = np.empty((B, N, D), np.float32)
    for b in range(B):
        outT = np.asarray(res.results[b]["outT"]).reshape(H, N)
        out[b] = outT.T
    return out, res


def _install_ntff_hook():
    import sys, types
    try:
        from antenv.axon_hooks import get_axon_ntff_profile_hook  # noqa
        return
    except ImportError:
        pass
    try:
        from trn_agent_boot.trn_boot import _ntff_profile_via_ctypes
        hook = _ntff_profile_via_ctypes("/opt/axon/libaxon_pjrt.so")
    except Exception:
        hook = None
    mod = types.ModuleType("antenv.axon_hooks")
    mod._hook = hook
    mod.get_axon_ntff_profile_hook = lambda: mod._hook
    mod.set_axon_ntff_profile_hook = lambda h: setattr(mod, "_hook", h)
    sys.modules["antenv.axon_hooks"] = mod


def kernel(**inputs):
    out, _ = run(inputs, trace=False)
    return out


# revision 18
# speedup vs baseline: 1.1183x; 1.0987x over previous
"""BiSSM global block on 8 Trainium2 NeuronCores.

Strategy (data-parallel over batch, one batch element per core):
  - Feature dims on SBUF partitions ("transposed" layout), time on the free
    axis: every einsum is a stationary-weight matmul with a wide moving dim.
  - The bidirectional linear recurrence h_t = h_{t-1} @ A.T + u_t has fast
    decaying memory (||(A.T)^k|| ~ 3e-3 at k=64): each direction is cut into
    S = N/K chunks of K steps, warmed up from zero over W extra steps.  All
    2S chunks advance together, so one scan step is a (H,H) matmul with a
    2S-wide moving operand - the whole scan is W+K matmul rounds instead of
    2N vector-matrix products.
  - u and y live in a (phase, chunk) factored layout: the per-step strided
    gathers become contiguous runs for the vector engine.
  - Backward scan relabels chunks c' = S-1-c so all reversed accesses are
    forward-strided in the same u buffer.
  - Matmul inputs bf16 (fp32 PSUM accumulate); y accumulated fp32.
  - Host prep (cheap, O(H^2)): assemble A, transpose weights, fold RMS
    `scale` and sqrt(D) into Bm/gate_w, and 0.5*alpha into Cm.
"""
import numpy as np
import ml_dtypes

B, N, D = 8, 2048, 768
H = 768
PT = 128          # SBUF partitions
KT = H // PT      # 6 feature tiles
K = 32            # chunk (emit) length
W = 32            # warmup window (multiple of K)
S = N // K        # 64 chunks per direction
CC = (N + 2 * W) // K   # 68 chunk-columns in padded u (2 pad + 64 + 2 pad)
NBLK = 512        # time-block for projections
NB = N // NBLK    # 4 blocks
EPS = 1e-8

_CACHE = {}


def _build_nc():
    import concourse.tile as tile
    from concourse import bacc, mybir
    from contextlib import ExitStack

    f32 = mybir.dt.float32
    bf16 = mybir.dt.bfloat16
    ACT = mybir.ActivationFunctionType

    nc = bacc.Bacc("TRN2", target_bir_lowering=False, debug=False, num_devices=8)

    xT_d = nc.dram_tensor("xT", [KT, PT, N], f32, kind="ExternalInput").ap()
    Mw_d = nc.dram_tensor("Mw", [KT, PT, H], bf16, kind="ExternalInput").ap()
    M2_d = nc.dram_tensor("M2", [KT, PT, H], bf16, kind="ExternalInput").ap()
    MC_d = nc.dram_tensor("MC", [KT, PT, D], bf16, kind="ExternalInput").ap()
    BC_d = nc.dram_tensor("BC", [KT, PT, D], bf16, kind="ExternalInput").ap()
    CmT_d = nc.dram_tensor("CmT", [KT, PT, D], bf16, kind="ExternalInput").ap()
    BmT_d = nc.dram_tensor("BmT", [KT, PT, H], bf16, kind="ExternalInput").ap()
    gwT_d = nc.dram_tensor("gwT", [KT, PT, D], bf16, kind="ExternalInput").ap()
    gb_d = nc.dram_tensor("gb", [PT, KT], f32, kind="ExternalInput").ap()
    outT_d = nc.dram_tensor("outT", [KT, PT, N], f32, kind="ExternalOutput").ap()

    with tile.TileContext(nc) as tc:
        with ExitStack() as ctx:
            persist = ctx.enter_context(tc.tile_pool(name="persist", bufs=1))
            stream = ctx.enter_context(tc.tile_pool(name="stream", bufs=2))
            scr1 = ctx.enter_context(tc.tile_pool(name="scr1", bufs=1))
            rpool = ctx.enter_context(tc.tile_pool(name="rpool", bufs=2))

            # ---- persistent SBUF ----
            # u2[p, j, phase, cc] <-> padded-u column cc*K + phase
            u2 = persist.tile([PT, KT, K, CC], bf16)
            g_sb = persist.tile([PT, KT, N], bf16)
            # y2[p, j, phase, c] <-> y column c*K + phase
            y2 = persist.tile([PT, KT, K, S], f32)
            gb_s = persist.tile([PT, KT], f32)
            ones_s = persist.tile([PT, PT], bf16)
            eps_s = persist.tile([PT, 1], f32)

            nc.gpsimd.dma_start(out=gb_s, in_=gb_d)
            nc.vector.memset(ones_s, 1.0)
            nc.vector.memset(eps_s, D * EPS)
            # zero u padding: W//K chunk-cols each side
            nc.vector.memset(u2[:, :, :, 0:W // K], 0.0)
            nc.vector.memset(u2[:, :, :, CC - W // K:CC], 0.0)

            # ---- phase 1: RMS norm, u-projection, gate (per 512-block) ----
            with tc.tile_pool(name="ph1psum", bufs=2, space="PSUM") as ph1p, \
                 tc.tile_pool(name="wph1", bufs=1) as wph1:
                BmT_s = wph1.tile([PT, KT, H], bf16)
                gwT_s = wph1.tile([PT, KT, D], bf16)
                BC_s = wph1.tile([PT, KT, D], bf16)
                for k in range(KT):
                    nc.gpsimd.dma_start(out=BmT_s[:, k, :], in_=BmT_d[k])
                    nc.gpsimd.dma_start(out=gwT_s[:, k, :], in_=gwT_d[k])
                    nc.gpsimd.dma_start(out=BC_s[:, k, :], in_=BC_d[k])
                for nb in range(NB):
                    cols = slice(nb * NBLK, (nb + 1) * NBLK)
                    xb = stream.tile([PT, KT, NBLK], f32, tag="xb")
                    for k in range(KT):
                        nc.gpsimd.dma_start(out=xb[:, k, :], in_=xT_d[k, :, cols])
                    # sumsq over d via all-ones matmul (broadcasts to all parts)
                    psR = ph1p.tile([PT, NBLK], f32, tag="psR")
                    sq = scr1.tile([PT, KT, NBLK], bf16, tag="sq")
                    for k in range(KT):
                        nc.vector.tensor_mul(sq[:, k, :], xb[:, k, :], xb[:, k, :])
                        nc.tensor.matmul(psR, ones_s, sq[:, k, :],
                                         start=(k == 0), stop=(k == KT - 1))
                    # rb = sqrt(sumsq + D*eps); 1/sqrt(D) folded into Bm/gw
                    rb = stream.tile([PT, NBLK], f32, tag="rb")
                    nc.scalar.activation(out=rb, in_=psR, func=ACT.Sqrt,
                                         bias=eps_s, scale=1.0)
                    nc.vector.reciprocal(out=rb, in_=rb)
                    xn = stream.tile([PT, KT, NBLK], bf16, tag="xn")
                    for k in range(KT):
                        nc.vector.tensor_mul(xn[:, k, :], xb[:, k, :], rb)
                    # u block -> u2 chunk-cols [nb*16+2, nb*16+18)
                    cc0 = nb * (NBLK // K) + W // K
                    for j in range(KT):
                        psU = ph1p.tile([PT, NBLK], f32, tag="psU")
                        for k in range(KT):
                            nc.tensor.matmul(psU, BmT_s[:, k, j * PT:(j + 1) * PT],
                                             xn[:, k, :],
                                             start=(k == 0), stop=(k == KT - 1))
                        dest = u2[:, j, :, cc0:cc0 + NBLK // K].transpose([0, 2, 1])
                        if j % 2 == 0:
                            nc.vector.tensor_copy(dest, psU)
                        else:
                            nc.scalar.activation(out=dest, in_=psU, func=ACT.Copy)
                    # UC = xn @ (Bm' Cm^T a/2): initializes y2 (the one
                    # MC-emitted contribution per column carries this term)
                    for j in range(KT):
                        psUC = ph1p.tile([PT, NBLK], f32, tag="psUC")
                        for k in range(KT):
                            nc.tensor.matmul(psUC, BC_s[:, k, j * PT:(j + 1) * PT],
                                             xn[:, k, :],
                                             start=(k == 0), stop=(k == KT - 1))
                        ydest = y2[:, j, :, nb * (NBLK // K):(nb + 1) * (NBLK // K)]
                        ydest = ydest.transpose([0, 2, 1])
                        if j % 2 == 0:
                            nc.scalar.activation(out=ydest, in_=psUC, func=ACT.Copy)
                        else:
                            nc.vector.tensor_copy(ydest, psUC)
                    # gate
                    for j in range(KT):
                        psG = ph1p.tile([PT, NBLK], f32, tag="psG")
                        for k in range(KT):
                            nc.tensor.matmul(psG, gwT_s[:, k, j * PT:(j + 1) * PT],
                                             xn[:, k, :],
                                             start=(k == 0), stop=(k == KT - 1))
                        nc.scalar.activation(out=g_sb[:, j, cols], in_=psG,
                                             func=ACT.Sigmoid, bias=gb_s[:, j:j + 1])

            # ---- phase 1.5: v = u@M; merge w into u2 (pair-step inputs) ----
            # fwd pair input at even padded col t: w[t] = v[t-1] + u[t]
            # bwd pair input at odd  padded col t: w~[t] = v[t+1] + u[t]
            # phases 0 and 31 stay raw in u2 (R0 init); their merged values
            # live in side buffers w0 / w31.
            w0 = persist.tile([PT, KT, CC], bf16)
            w31 = persist.tile([PT, KT, CC], bf16)
            NPH = 4   # phases per v-matmul group
            with tc.tile_pool(name="vstage", bufs=1) as vst, \
                 tc.tile_pool(name="vpsum", bufs=2, space="PSUM") as vps:
                Mw_s = vst.tile([PT, KT, H], bf16)
                for k in range(KT):
                    nc.gpsimd.dma_start(out=Mw_s[:, k, :], in_=Mw_d[k])
                vbuf = vst.tile([PT, KT, K, CC], bf16)
                for gph in range(K // NPH):
                    phs = slice(gph * NPH, (gph + 1) * NPH)
                    for j in range(KT):
                        psV = vps.tile([PT, NPH, CC], f32, tag="psV")
                        for k in range(KT):
                            nc.tensor.matmul(psV, Mw_s[:, k, j * PT:(j + 1) * PT],
                                             u2[:, k, phs, :],
                                             start=(k == 0), stop=(k == KT - 1))
                        nc.scalar.activation(out=vbuf[:, j, phs, :], in_=psV,
                                             func=ACT.Copy)
                # merges (in-place into u2 for phases 1..30)
                for j in range(KT):
                    # even phases 2..30: w = v[p-1] + u[p]
                    tgt = u2[:, j, 2:K - 1:2, :]
                    nc.vector.tensor_add(tgt, vbuf[:, j, 1:K - 2:2, :], tgt)
                    # odd phases 1..29: w~ = v[p+1] + u[p]
                    tgt = u2[:, j, 1:K - 2:2, :]
                    nc.vector.tensor_add(tgt, vbuf[:, j, 2:K - 1:2, :], tgt)
                    # w0[cc] = v[31, cc-1] + u[0, cc]
                    nc.vector.memset(w0[:, j, 0:1], 0.0)
                    nc.vector.tensor_add(w0[:, j, 1:CC], vbuf[:, j, K - 1, 0:CC - 1],
                                         u2[:, j, 0, 1:CC])
                    # w31[cc] = v[0, cc+1] + u[31, cc]
                    nc.vector.memset(w31[:, j, CC - 1:CC], 0.0)
                    nc.vector.tensor_add(w31[:, j, 0:CC - 1], vbuf[:, j, 0, 1:CC],
                                         u2[:, j, K - 1, 0:CC - 1])

            # ---- phase 2: paired (M^2) bidirectional recurrence ----
            # States at even local steps only.  Round to state s (even):
            #   fwd chunk c consumes slot col c*K + s, bwd col c*K + (2W+K-1-s)
            # Emits (s >= W, i = s-W even):
            #   direct (Cm^T): fwd -> y2 phase i,   bwd -> phase K-1-i
            #   MC (M Cm^T):   fwd -> y2 phase i+1, bwd -> phase K-2-i
            with tc.tile_pool(name="ph2psum", bufs=2, space="PSUM") as ph2p, \
                 tc.tile_pool(name="ph2psumC", bufs=3, space="PSUM") as ph2pc, \
                 tc.tile_pool(name="wph2", bufs=1) as wph2:
                M2_s = wph2.tile([PT, KT, H], bf16)
                CmT_s = wph2.tile([PT, KT, D], bf16)
                MC_s = wph2.tile([PT, KT, D], bf16)
                for k in range(KT):
                    nc.gpsimd.dma_start(out=M2_s[:, k, :], in_=M2_d[k])
                    nc.gpsimd.dma_start(out=CmT_s[:, k, :], in_=CmT_d[k])
                    nc.gpsimd.dma_start(out=MC_s[:, k, :], in_=MC_d[k])
                R_prev = rpool.tile([PT, KT, 2 * S], bf16, tag="R")
                nc.vector.tensor_copy(R_prev[:, :, 0:S], u2[:, :, 0, 0:S])
                nc.vector.tensor_copy(R_prev[:, :, S:2 * S],
                                      u2[:, :, K - 1, (2 * W) // K:(2 * W) // K + S])
                for s in range(2, W + K, 2):
                    q, r = divmod(s, K)
                    fwd_u = w0[:, :, q:q + S] if r == 0 else u2[:, :, r, q:q + S]
                    cb = 2 * W + K - 1 - s
                    qb, rb_ = divmod(cb, K)
                    bwd_u = (w31[:, :, qb:qb + S] if rb_ == K - 1
                             else u2[:, :, rb_, qb:qb + S])
                    R_new = rpool.tile([PT, KT, 2 * S], bf16, tag="R")
                    psA4 = ph2p.tile([PT, 4, 2 * S], f32, tag="psA4")
                    psA2 = ph2p.tile([PT, 2, 2 * S], f32, tag="psA2")
                    for j in range(KT):
                        ps = psA4[:, j, :] if j < 4 else psA2[:, j - 4, :]
                        for k in range(KT):
                            nc.tensor.matmul(ps, M2_s[:, k, j * PT:(j + 1) * PT],
                                             R_prev[:, k, :],
                                             start=(k == 0), stop=(k == KT - 1))
                    nc.vector.tensor_add(R_new[:, 0:4, 0:S], psA4[:, :, 0:S],
                                         fwd_u[:, 0:4, :])
                    nc.vector.tensor_add(R_new[:, 0:4, S:2 * S],
                                         psA4[:, :, S:2 * S], bwd_u[:, 0:4, :])
                    nc.vector.tensor_add(R_new[:, 4:6, 0:S], psA2[:, :, 0:S],
                                         fwd_u[:, 4:6, :])
                    nc.vector.tensor_add(R_new[:, 4:6, S:2 * S],
                                         psA2[:, :, S:2 * S], bwd_u[:, 4:6, :])
                    if s >= W:
                        i = s - W
                        for wt, pf, pb in ((CmT_s, i, K - 1 - i),
                                           (MC_s, i + 1, K - 2 - i)):
                            for dg in range(2):
                                psC = ph2pc.tile([PT, 3, 2 * S], f32, tag="psC")
                                for dd in range(3):
                                    j = dg * 3 + dd
                                    for k in range(KT):
                                        nc.tensor.matmul(
                                            psC[:, dd, :],
                                            wt[:, k, j * PT:(j + 1) * PT],
                                            R_new[:, k, :],
                                            start=(k == 0), stop=(k == KT - 1))
                                yf = y2[:, dg * 3:(dg + 1) * 3, pf, :]
                                nc.vector.tensor_add(yf, yf, psC[:, :, 0:S])
                                yb = y2[:, dg * 3:(dg + 1) * 3, pb, :]
                                nc.vector.tensor_add(yb, yb, psC[:, :, S:2 * S])
                    R_prev = R_new

            # ---- phase 3: out = x + y*g  (y pre-scaled by 0.5*alpha) ----
            for nb in range(NB):
                cols = slice(nb * NBLK, (nb + 1) * NBLK)
                xb = stream.tile([PT, KT, NBLK], f32, tag="xb")
                for k in range(KT):
                    nc.gpsimd.dma_start(out=xb[:, k, :], in_=xT_d[k, :, cols])
                # y2 block in t-order: (cc-major, phase-minor)
                c0 = nb * (NBLK // K)
                ysl = y2[:, :, :, c0:c0 + NBLK // K].transpose([0, 1, 3, 2])
                yg = scr1.tile([PT, KT, NBLK // K, K], f32, tag="yg")
                gsl = g_sb[:, :, cols].rearrange("p j (c f) -> p j c f", f=K)
                nc.vector.tensor_mul(yg, ysl, gsl)
                nc.vector.tensor_add(yg, yg, xb.rearrange("p j (c f) -> p j c f", f=K))
                for k in range(KT):
                    nc.gpsimd.dma_start(
                        out=outT_d[k, :, cols],
                        in_=yg[:, k, :, :].rearrange("p c f -> p (c f)"))

    nc.compile()
    return nc


def _get_nc():
    if "nc" not in _CACHE:
        _CACHE["nc"] = _build_nc()
    return _CACHE["nc"]


def _prep_maps(x, scale, U, V, S_param, Bm, Cm, gate_w, gate_b, alpha):
    bf = ml_dtypes.bfloat16
    A_diag = -np.linspace(1.0, float(H), H, dtype=np.float32) / H
    A = np.diag(A_diag) + U @ V.T + (S_param - S_param.T)
    M = np.ascontiguousarray(A.T)                       # (h_in, h_out)
    CmT = np.ascontiguousarray(Cm.T) * (0.5 * float(alpha[0]))
    sD = np.sqrt(float(D)).astype(np.float32)
    BmT = np.ascontiguousarray(Bm.T) * (scale * sD)[:, None]
    gwT = np.ascontiguousarray(gate_w.T) * (scale * sD)[:, None]

    def tiles(w):  # (H, F) -> (KT, PT, F)
        return np.ascontiguousarray(w.reshape(KT, PT, -1))

    M64 = M.astype(np.float64)
    M2 = (M64 @ M64).astype(np.float32)
    MC = (M64 @ CmT.astype(np.float64)).astype(np.float32)
    BC = (BmT.astype(np.float64) @ CmT.astype(np.float64)).astype(np.float32)
    Mw = tiles(M).astype(bf)
    M2s = tiles(M2).astype(bf)
    MCs = tiles(MC).astype(bf)
    BCs = tiles(BC).astype(bf)
    CmTs = tiles(CmT).astype(bf)
    BmTs = tiles(BmT).astype(bf)
    gwTs = tiles(gwT).astype(bf)
    gb = np.ascontiguousarray(gate_b.reshape(KT, PT).T).astype(np.float32)

    in_maps = []
    for b in range(B):
        xT = np.ascontiguousarray(x[b].T.reshape(KT, PT, N)).astype(np.float32)
        in_maps.append({"xT": xT, "Mw": Mw, "M2": M2s, "MC": MCs, "BC": BCs,
                        "CmT": CmTs, "BmT": BmTs, "gwT": gwTs, "gb": gb})
    return in_maps


def run(inputs, trace=False):
    from concourse.bass_utils import run_bass_kernel_spmd
    nc = _get_nc()
    in_maps = _prep_maps(**inputs)
    if trace:
        _install_ntff_hook()
    res = run_bass_kernel_spmd(nc, in_maps, core_ids=list(range(8)), trace=trace)
    out = np.empty((B, N, D), np.float32)
    for b in range(B):
        outT = np.asarray(res.results[b]["outT"]).reshape(H, N)
        out[b] = outT.T
    return out, res


def _install_ntff_hook():
    import sys, types
    try:
        from antenv.axon_hooks import get_axon_ntff_profile_hook  # noqa
        return
    except ImportError:
        pass
    try:
        from trn_agent_boot.trn_boot import _ntff_profile_via_ctypes
        hook = _ntff_profile_via_ctypes("/opt/axon/libaxon_pjrt.so")
    except Exception:
        hook = None
    mod = types.ModuleType("antenv.axon_hooks")
    mod._hook = hook
    mod.get_axon_ntff_profile_hook = lambda: mod._hook
    mod.set_axon_ntff_profile_hook = lambda h: setattr(mod, "_hook", h)
    sys.modules["antenv.axon_hooks"] = mod


def kernel(**inputs):
    out, _ = run(inputs, trace=False)
    return out


# revision 19
# speedup vs baseline: 1.1220x; 1.0033x over previous
"""BiSSM global block on 8 Trainium2 NeuronCores.

Strategy (data-parallel over batch, one batch element per core):
  - Feature dims on SBUF partitions ("transposed" layout), time on the free
    axis: every einsum is a stationary-weight matmul with a wide moving dim.
  - The bidirectional linear recurrence h_t = h_{t-1} @ A.T + u_t has fast
    decaying memory (||(A.T)^k|| ~ 3e-3 at k=64): each direction is cut into
    S = N/K chunks of K steps, warmed up from zero over W extra steps.  All
    2S chunks advance together, so one scan step is a (H,H) matmul with a
    2S-wide moving operand - the whole scan is W+K matmul rounds instead of
    2N vector-matrix products.
  - u and y live in a (phase, chunk) factored layout: the per-step strided
    gathers become contiguous runs for the vector engine.
  - Backward scan relabels chunks c' = S-1-c so all reversed accesses are
    forward-strided in the same u buffer.
  - Matmul inputs bf16 (fp32 PSUM accumulate); y accumulated fp32.
  - Host prep (cheap, O(H^2)): assemble A, transpose weights, fold RMS
    `scale` and sqrt(D) into Bm/gate_w, and 0.5*alpha into Cm.
"""
import numpy as np
import ml_dtypes

B, N, D = 8, 2048, 768
H = 768
PT = 128          # SBUF partitions
KT = H // PT      # 6 feature tiles
K = 32            # chunk (emit) length
W = 32            # warmup window (multiple of K)
S = N // K        # 64 chunks per direction
CC = (N + 2 * W) // K   # 68 chunk-columns in padded u (2 pad + 64 + 2 pad)
NBLK = 512        # time-block for projections
NB = N // NBLK    # 4 blocks
EPS = 1e-8

_CACHE = {}


def _build_nc():
    import concourse.tile as tile
    from concourse import bacc, mybir
    from contextlib import ExitStack

    f32 = mybir.dt.float32
    bf16 = mybir.dt.bfloat16
    ACT = mybir.ActivationFunctionType

    nc = bacc.Bacc("TRN2", target_bir_lowering=False, debug=False, num_devices=8)

    xT_d = nc.dram_tensor("xT", [KT, PT, N], f32, kind="ExternalInput").ap()
    Mw_d = nc.dram_tensor("Mw", [KT, PT, H], bf16, kind="ExternalInput").ap()
    M2_d = nc.dram_tensor("M2", [KT, PT, H], bf16, kind="ExternalInput").ap()
    MC_d = nc.dram_tensor("MC", [KT, PT, D], bf16, kind="ExternalInput").ap()
    BC_d = nc.dram_tensor("BC", [KT, PT, D], bf16, kind="ExternalInput").ap()
    CmT_d = nc.dram_tensor("CmT", [KT, PT, D], bf16, kind="ExternalInput").ap()
    BmT_d = nc.dram_tensor("BmT", [KT, PT, H], bf16, kind="ExternalInput").ap()
    gwT_d = nc.dram_tensor("gwT", [KT, PT, D], bf16, kind="ExternalInput").ap()
    gb_d = nc.dram_tensor("gb", [PT, KT], f32, kind="ExternalInput").ap()
    outT_d = nc.dram_tensor("outT", [KT, PT, N], f32, kind="ExternalOutput").ap()

    with tile.TileContext(nc) as tc:
        with ExitStack() as ctx:
            persist = ctx.enter_context(tc.tile_pool(name="persist", bufs=1))
            stream = ctx.enter_context(tc.tile_pool(name="stream", bufs=2))
            scr1 = ctx.enter_context(tc.tile_pool(name="scr1", bufs=1))
            rpool = ctx.enter_context(tc.tile_pool(name="rpool", bufs=3))

            # ---- persistent SBUF ----
            # u2[p, j, phase, cc] <-> padded-u column cc*K + phase
            u2 = persist.tile([PT, KT, K, CC], bf16)
            g_sb = persist.tile([PT, KT, N], bf16)
            # y2[p, j, phase, c] <-> y column c*K + phase
            y2 = persist.tile([PT, KT, K, S], f32)
            gb_s = persist.tile([PT, KT], f32)
            ones_s = persist.tile([PT, PT], bf16)
            eps_s = persist.tile([PT, 1], f32)

            nc.gpsimd.dma_start(out=gb_s, in_=gb_d)
            nc.vector.memset(ones_s, 1.0)
            nc.vector.memset(eps_s, D * EPS)
            # zero u padding: W//K chunk-cols each side
            nc.vector.memset(u2[:, :, :, 0:W // K], 0.0)
            nc.vector.memset(u2[:, :, :, CC - W // K:CC], 0.0)

            # ---- phase 1: RMS norm, u-projection, gate (per 512-block) ----
            with tc.tile_pool(name="ph1psum", bufs=2, space="PSUM") as ph1p, \
                 tc.tile_pool(name="wph1", bufs=1) as wph1:
                BmT_s = wph1.tile([PT, KT, H], bf16)
                gwT_s = wph1.tile([PT, KT, D], bf16)
                BC_s = wph1.tile([PT, KT, D], bf16)
                for k in range(KT):
                    nc.gpsimd.dma_start(out=BmT_s[:, k, :], in_=BmT_d[k])
                    nc.gpsimd.dma_start(out=gwT_s[:, k, :], in_=gwT_d[k])
                    nc.gpsimd.dma_start(out=BC_s[:, k, :], in_=BC_d[k])
                for nb in range(NB):
                    cols = slice(nb * NBLK, (nb + 1) * NBLK)
                    xb = stream.tile([PT, KT, NBLK], f32, tag="xb")
                    for k in range(KT):
                        nc.gpsimd.dma_start(out=xb[:, k, :], in_=xT_d[k, :, cols])
                    # sumsq over d via all-ones matmul (broadcasts to all parts)
                    psR = ph1p.tile([PT, NBLK], f32, tag="psR")
                    sq = scr1.tile([PT, KT, NBLK], bf16, tag="sq")
                    for k in range(KT):
                        nc.vector.tensor_mul(sq[:, k, :], xb[:, k, :], xb[:, k, :])
                        nc.tensor.matmul(psR, ones_s, sq[:, k, :],
                                         start=(k == 0), stop=(k == KT - 1))
                    # rb = sqrt(sumsq + D*eps); 1/sqrt(D) folded into Bm/gw
                    rb = stream.tile([PT, NBLK], f32, tag="rb")
                    nc.scalar.activation(out=rb, in_=psR, func=ACT.Sqrt,
                                         bias=eps_s, scale=1.0)
                    nc.vector.reciprocal(out=rb, in_=rb)
                    xn = stream.tile([PT, KT, NBLK], bf16, tag="xn")
                    for k in range(KT):
                        nc.vector.tensor_mul(xn[:, k, :], xb[:, k, :], rb)
                    # u block -> u2 chunk-cols [nb*16+2, nb*16+18)
                    cc0 = nb * (NBLK // K) + W // K
                    for j in range(KT):
                        psU = ph1p.tile([PT, NBLK], f32, tag="psU")
                        for k in range(KT):
                            nc.tensor.matmul(psU, BmT_s[:, k, j * PT:(j + 1) * PT],
                                             xn[:, k, :],
                                             start=(k == 0), stop=(k == KT - 1))
                        dest = u2[:, j, :, cc0:cc0 + NBLK // K].transpose([0, 2, 1])
                        if j % 2 == 0:
                            nc.vector.tensor_copy(dest, psU)
                        else:
                            nc.scalar.activation(out=dest, in_=psU, func=ACT.Copy)
                    # UC = xn @ (Bm' Cm^T a/2): initializes y2 (the one
                    # MC-emitted contribution per column carries this term)
                    for j in range(KT):
                        psUC = ph1p.tile([PT, NBLK], f32, tag="psUC")
                        for k in range(KT):
                            nc.tensor.matmul(psUC, BC_s[:, k, j * PT:(j + 1) * PT],
                                             xn[:, k, :],
                                             start=(k == 0), stop=(k == KT - 1))
                        ydest = y2[:, j, :, nb * (NBLK // K):(nb + 1) * (NBLK // K)]
                        ydest = ydest.transpose([0, 2, 1])
                        if j % 2 == 0:
                            nc.scalar.activation(out=ydest, in_=psUC, func=ACT.Copy)
                        else:
                            nc.vector.tensor_copy(ydest, psUC)
                    # gate
                    for j in range(KT):
                        psG = ph1p.tile([PT, NBLK], f32, tag="psG")
                        for k in range(KT):
                            nc.tensor.matmul(psG, gwT_s[:, k, j * PT:(j + 1) * PT],
                                             xn[:, k, :],
                                             start=(k == 0), stop=(k == KT - 1))
                        nc.scalar.activation(out=g_sb[:, j, cols], in_=psG,
                                             func=ACT.Sigmoid, bias=gb_s[:, j:j + 1])

            # ---- phase 1.5: v = u@M; merge w into u2 (pair-step inputs) ----
            # fwd pair input at even padded col t: w[t] = v[t-1] + u[t]
            # bwd pair input at odd  padded col t: w~[t] = v[t+1] + u[t]
            # phases 0 and 31 stay raw in u2 (R0 init); their merged values
            # live in side buffers w0 / w31.
            w0 = persist.tile([PT, KT, CC], bf16)
            w31 = persist.tile([PT, KT, CC], bf16)
            NPH = 4   # phases per v-matmul group
            with tc.tile_pool(name="vstage", bufs=1) as vst, \
                 tc.tile_pool(name="vpsum", bufs=2, space="PSUM") as vps:
                Mw_s = vst.tile([PT, KT, H], bf16)
                for k in range(KT):
                    nc.gpsimd.dma_start(out=Mw_s[:, k, :], in_=Mw_d[k])
                vbuf = vst.tile([PT, KT, K, CC], bf16)
                for gph in range(K // NPH):
                    phs = slice(gph * NPH, (gph + 1) * NPH)
                    for j in range(KT):
                        psV = vps.tile([PT, NPH, CC], f32, tag="psV")
                        for k in range(KT):
                            nc.tensor.matmul(psV, Mw_s[:, k, j * PT:(j + 1) * PT],
                                             u2[:, k, phs, :],
                                             start=(k == 0), stop=(k == KT - 1))
                        nc.scalar.activation(out=vbuf[:, j, phs, :], in_=psV,
                                             func=ACT.Copy)
                # merges (in-place into u2 for phases 1..30)
                for j in range(KT):
                    # even phases 2..30: w = v[p-1] + u[p]
                    tgt = u2[:, j, 2:K - 1:2, :]
                    nc.vector.tensor_add(tgt, vbuf[:, j, 1:K - 2:2, :], tgt)
                    # odd phases 1..29: w~ = v[p+1] + u[p]
                    tgt = u2[:, j, 1:K - 2:2, :]
                    nc.vector.tensor_add(tgt, vbuf[:, j, 2:K - 1:2, :], tgt)
                    # w0[cc] = v[31, cc-1] + u[0, cc]
                    nc.vector.memset(w0[:, j, 0:1], 0.0)
                    nc.vector.tensor_add(w0[:, j, 1:CC], vbuf[:, j, K - 1, 0:CC - 1],
                                         u2[:, j, 0, 1:CC])
                    # w31[cc] = v[0, cc+1] + u[31, cc]
                    nc.vector.memset(w31[:, j, CC - 1:CC], 0.0)
                    nc.vector.tensor_add(w31[:, j, 0:CC - 1], vbuf[:, j, 0, 1:CC],
                                         u2[:, j, K - 1, 0:CC - 1])

            # ---- phase 2: paired (M^2) bidirectional recurrence ----
            # States at even local steps only.  Round to state s (even):
            #   fwd chunk c consumes slot col c*K + s, bwd col c*K + (2W+K-1-s)
            # Emits (s >= W, i = s-W even):
            #   direct (Cm^T): fwd -> y2 phase i,   bwd -> phase K-1-i
            #   MC (M Cm^T):   fwd -> y2 phase i+1, bwd -> phase K-2-i
            with tc.tile_pool(name="ph2psum", bufs=2, space="PSUM") as ph2p, \
                 tc.tile_pool(name="ph2psumC", bufs=4, space="PSUM") as ph2pc, \
                 tc.tile_pool(name="wph2", bufs=1) as wph2:
                M2_s = wph2.tile([PT, KT, H], bf16)
                CmT_s = wph2.tile([PT, KT, D], bf16)
                MC_s = wph2.tile([PT, KT, D], bf16)
                for k in range(KT):
                    nc.gpsimd.dma_start(out=M2_s[:, k, :], in_=M2_d[k])
                    nc.gpsimd.dma_start(out=CmT_s[:, k, :], in_=CmT_d[k])
                    nc.gpsimd.dma_start(out=MC_s[:, k, :], in_=MC_d[k])
                R_prev = rpool.tile([PT, KT, 2 * S], bf16, tag="R")
                nc.vector.tensor_copy(R_prev[:, :, 0:S], u2[:, :, 0, 0:S])
                nc.vector.tensor_copy(R_prev[:, :, S:2 * S],
                                      u2[:, :, K - 1, (2 * W) // K:(2 * W) // K + S])
                for s in range(2, W + K, 2):
                    q, r = divmod(s, K)
                    fwd_u = w0[:, :, q:q + S] if r == 0 else u2[:, :, r, q:q + S]
                    cb = 2 * W + K - 1 - s
                    qb, rb_ = divmod(cb, K)
                    bwd_u = (w31[:, :, qb:qb + S] if rb_ == K - 1
                             else u2[:, :, rb_, qb:qb + S])
                    R_new = rpool.tile([PT, KT, 2 * S], bf16, tag="R")
                    psA4 = ph2p.tile([PT, 4, 2 * S], f32, tag="psA4")
                    psA2 = ph2p.tile([PT, 2, 2 * S], f32, tag="psA2")
                    for j in range(KT):
                        ps = psA4[:, j, :] if j < 4 else psA2[:, j - 4, :]
                        for k in range(KT):
                            nc.tensor.matmul(ps, M2_s[:, k, j * PT:(j + 1) * PT],
                                             R_prev[:, k, :],
                                             start=(k == 0), stop=(k == KT - 1))
                    nc.vector.tensor_add(R_new[:, 0:4, 0:S], psA4[:, :, 0:S],
                                         fwd_u[:, 0:4, :])
                    nc.vector.tensor_add(R_new[:, 0:4, S:2 * S],
                                         psA4[:, :, S:2 * S], bwd_u[:, 0:4, :])
                    nc.vector.tensor_add(R_new[:, 4:6, 0:S], psA2[:, :, 0:S],
                                         fwd_u[:, 4:6, :])
                    nc.vector.tensor_add(R_new[:, 4:6, S:2 * S],
                                         psA2[:, :, S:2 * S], bwd_u[:, 4:6, :])
                    if s >= W:
                        i = s - W
                        for wt, pf, pb in ((CmT_s, i, K - 1 - i),
                                           (MC_s, i + 1, K - 2 - i)):
                            for dg in range(2):
                                psC = ph2pc.tile([PT, 3, 2 * S], f32, tag="psC")
                                for dd in range(3):
                                    j = dg * 3 + dd
                                    for k in range(KT):
                                        nc.tensor.matmul(
                                            psC[:, dd, :],
                                            wt[:, k, j * PT:(j + 1) * PT],
                                            R_new[:, k, :],
                                            start=(k == 0), stop=(k == KT - 1))
                                yf = y2[:, dg * 3:(dg + 1) * 3, pf, :]
                                nc.vector.tensor_add(yf, yf, psC[:, :, 0:S])
                                yb = y2[:, dg * 3:(dg + 1) * 3, pb, :]
                                nc.vector.tensor_add(yb, yb, psC[:, :, S:2 * S])
                    R_prev = R_new

            # ---- phase 3: out = x + y*g  (y pre-scaled by 0.5*alpha) ----
            for nb in range(NB):
                cols = slice(nb * NBLK, (nb + 1) * NBLK)
                xb = stream.tile([PT, KT, NBLK], f32, tag="xb")
                for k in range(KT):
                    nc.gpsimd.dma_start(out=xb[:, k, :], in_=xT_d[k, :, cols])
                # y2 block in t-order: (cc-major, phase-minor)
                c0 = nb * (NBLK // K)
                ysl = y2[:, :, :, c0:c0 + NBLK // K].transpose([0, 1, 3, 2])
                yg = scr1.tile([PT, KT, NBLK // K, K], f32, tag="yg")
                gsl = g_sb[:, :, cols].rearrange("p j (c f) -> p j c f", f=K)
                nc.vector.tensor_mul(yg, ysl, gsl)
                nc.vector.tensor_add(yg, yg, xb.rearrange("p j (c f) -> p j c f", f=K))
                for k in range(KT):
                    nc.gpsimd.dma_start(
                        out=outT_d[k, :, cols],
                        in_=yg[:, k, :, :].rearrange("p c f -> p (c f)"))

    nc.compile()
    return nc


def _get_nc():
    if "nc" not in _CACHE:
        _CACHE["nc"] = _build_nc()
    return _CACHE["nc"]


def _prep_maps(x, scale, U, V, S_param, Bm, Cm, gate_w, gate_b, alpha):
    bf = ml_dtypes.bfloat16
    A_diag = -np.linspace(1.0, float(H), H, dtype=np.float32) / H
    A = np.diag(A_diag) + U @ V.T + (S_param - S_param.T)
    M = np.ascontiguousarray(A.T)                       # (h_in, h_out)
    CmT = np.ascontiguousarray(Cm.T) * (0.5 * float(alpha[0]))
    sD = np.sqrt(float(D)).astype(np.float32)
    BmT = np.ascontiguousarray(Bm.T) * (scale * sD)[:, None]
    gwT = np.ascontiguousarray(gate_w.T) * (scale * sD)[:, None]

    def tiles(w):  # (H, F) -> (KT, PT, F)
        return np.ascontiguousarray(w.reshape(KT, PT, -1))

    M64 = M.astype(np.float64)
    M2 = (M64 @ M64).astype(np.float32)
    MC = (M64 @ CmT.astype(np.float64)).astype(np.float32)
    BC = (BmT.astype(np.float64) @ CmT.astype(np.float64)).astype(np.float32)
    Mw = tiles(M).astype(bf)
    M2s = tiles(M2).astype(bf)
    MCs = tiles(MC).astype(bf)
    BCs = tiles(BC).astype(bf)
    CmTs = tiles(CmT).astype(bf)
    BmTs = tiles(BmT).astype(bf)
    gwTs = tiles(gwT).astype(bf)
    gb = np.ascontiguousarray(gate_b.reshape(KT, PT).T).astype(np.float32)

    in_maps = []
    for b in range(B):
        xT = np.ascontiguousarray(x[b].T.reshape(KT, PT, N)).astype(np.float32)
        in_maps.append({"xT": xT, "Mw": Mw, "M2": M2s, "MC": MCs, "BC": BCs,
                        "CmT": CmTs, "BmT": BmTs, "gwT": gwTs, "gb": gb})
    return in_maps


def run(inputs, trace=False):
    from concourse.bass_utils import run_bass_kernel_spmd
    nc = _get_nc()
    in_maps = _prep_maps(**inputs)
    if trace:
        _install_ntff_hook()
    res = run_bass_kernel_spmd(nc, in_maps, core_ids=list(range(8)), trace=trace)
    out = np.empty((B, N, D), np.float32)
    for b in range(B):
        outT = np.asarray(res.results[b]["outT"]).reshape(H, N)
        out[b] = outT.T
    return out, res


def _install_ntff_hook():
    import sys, types
    try:
        from antenv.axon_hooks import get_axon_ntff_profile_hook  # noqa
        return
    except ImportError:
        pass
    try:
        from trn_agent_boot.trn_boot import _ntff_profile_via_ctypes
        hook = _ntff_profile_via_ctypes("/opt/axon/libaxon_pjrt.so")
    except Exception:
        hook = None
    mod = types.ModuleType("antenv.axon_hooks")
    mod._hook = hook
    mod.get_axon_ntff_profile_hook = lambda: mod._hook
    mod.set_axon_ntff_profile_hook = lambda h: setattr(mod, "_hook", h)
    sys.modules["antenv.axon_hooks"] = mod


def kernel(**inputs):
    out, _ = run(inputs, trace=False)
    return out
